# revision 14
# baseline (speedup 1.0000x reference)
"""GroupedQueryAttention (B=1, T=4096, D=2048, 16 heads / 4 kv heads, RoPE,
causal) on 8 Trainium2 NeuronCores.

Sharding: head tensor-parallel. Core c owns q-heads {2c, 2c+1} and kv head
c//2 (WQ/WO split along head dim, WK/WV along kv-head dim). Each core
computes its partial out = ctx_heads @ WO_slice over the full sequence;
partials are summed across cores on the host gather side.

v3 pipeline:
 - softmax denominator: per-pair DVE accumulation of P into a bf16 running
   sum + one ones-matrix matmul per (head, chunk) that also broadcasts the
   sums across partitions; reciprocal via ACT exp(-Ln(l)) (same table set
   as the softmax Exp, so no ACT table switches).
 - exp batched over key-tile PAIRS: one ACT instruction per [128,1024]
   PSUM S-pair (amortizes the 352-cycle ACT fixed overhead).
 - V produced transposed (weight-stationary chains, N=512 streams) then
   de-transposed with dma_start_transpose (XBAR).
 - flat cross-head pair stream with one-pair-deferred ctx emission and
   deferred per-head normalization, so the PE never drains at head or
   chunk boundaries (keeps HAM at full clock).
 - WO matmuls of chunk qc interleaved into chunk qc+1's attention stream;
   PSUM->SBUF output copies alternate between ACT and DVE.
 - split weight/x DMAs + 2-chunk-ahead prefetch to cut startup and SP
   queue head-of-line blocking; bf16 rope; bf16 partial-output DMA.
"""

import math

import numpy as np
import ml_dtypes

import concourse.bass as bass
import concourse.mybir as mybir
import concourse.tile as tile
from concourse.bass_utils import run_bass_kernel_spmd

FP = mybir.dt.float32
BF = mybir.dt.bfloat16
BFNP = ml_dtypes.bfloat16

T, D, DH = 4096, 2048, 128      # seq len, model dim, head dim
CH = 512                        # query-chunk (free dim of attention matmuls)
N_CORES = 8

nT = T // CH          # 8 q-chunks
nA = D // 128         # 16 contraction tiles
nS = CH // 128        # 4 128-subtiles per chunk
nN = D // 512         # 4 output column tiles
ISQ = 1.0 / math.sqrt(float(DH))


# --------------------------------------------------------------------------
# workaround: this walrus build rejects instructions carrying >1 sem-waits
# (setupSyncWait "Too many sync wait commands"); split extras into NoOps.
_WS_CTR = [0]


def _split_multi_waits(nc, limit=1):
    for f in nc.m.functions:
        for bb in f.blocks:
            il = bb.instructions
            i = 0
            while i < len(il):
                inst = il[i]
                si = getattr(inst, "sync_info", None)
                if si is not None and len(si.on_wait) > limit:
                    waits = list(si.on_wait)
                    keep, rest = waits[:limit], waits[limit:]
                    nops = []
                    for j in range(0, len(rest), limit):
                        _WS_CTR[0] += 1
                        n = mybir.InstNoOp(name=f"waitsplit-{_WS_CTR[0]}")
                        n.engine = inst.engine
                        n.sync_info = mybir.SyncInfo(
                            on_wait=rest[j:j + limit], on_update=[])
                        nops.append(n)
                    inst.sync_info = mybir.SyncInfo(
                        on_wait=keep, on_update=list(si.on_update))
                    for k, n in enumerate(nops):
                        il.insert(i + k, n)
                    i += len(nops)
                i += 1


# --------------------------------------------------------------------------
def build_nc():
    nc = bass.Bass()

    xT = nc.dram_tensor("xT", [D, T], BF, kind="ExternalInput")
    wq2 = nc.dram_tensor("wq2", [128, nA * 256], BF, kind="ExternalInput")
    wk2 = nc.dram_tensor("wk2", [128, nA * 128], BF, kind="ExternalInput")
    wv2 = nc.dram_tensor("wv2", [128, nA * 128], BF, kind="ExternalInput")
    wo2 = nc.dram_tensor("wo2", [128, 2 * D], BF, kind="ExternalInput")
    cosT = nc.dram_tensor("cosT", [128, T], BF, kind="ExternalInput")
    sinT = nc.dram_tensor("sinT", [128, T], BF, kind="ExternalInput")
    masks = nc.dram_tensor("masks", [128, nS * CH], BF, kind="ExternalInput")
    onesM = nc.dram_tensor("onesM", [128, 128], BF, kind="ExternalInput")
    out = nc.dram_tensor("out", [T, D], BF, kind="ExternalOutput")

    xTr = xT.rearrange("(a p) t -> p a t", p=128)

    with tile.TileContext(nc) as tc:
        with (
            tc.tile_pool(name="res", bufs=1) as res,
            tc.tile_pool(name="ktv", bufs=2 * nT) as ktv,
            tc.tile_pool(name="xt", bufs=3) as xtp,
            tc.tile_pool(name="tab", bufs=6) as tab,
            tc.tile_pool(name="work", bufs=2) as work,
            tc.tile_pool(name="pp", bufs=6) as pp,
            tc.tile_pool(name="accp", bufs=2) as accp,
            tc.tile_pool(name="psS", bufs=2, space="PSUM") as psS,
            tc.tile_pool(name="psC", bufs=2, space="PSUM") as psC,
            tc.tile_pool(name="psA", bufs=2, space="PSUM") as psA,
        ):
            wq_sb = res.tile([128, nA * 256], BF, name="wq_sb")
            wk_sb = res.tile([128, nA * 128], BF, name="wk_sb")
            wv_sb = res.tile([128, nA * 128], BF, name="wv_sb")
            wo_sb = res.tile([128, 2 * D], BF, name="wo_sb")
            mask_sb = res.tile([128, nS * CH], BF, name="mask_sb")
            ones_sb = res.tile([128, 128], BF, name="ones_sb")

            def wq_part(part):
                c0, c1 = part * nA * 64, (part + 1) * nA * 64
                nc.sync.dma_start(wq_sb[:, c0:c1], wq2[:, c0:c1])

            def wkv_part(w_sb, w_dram, part):
                c0, c1 = part * nA * 64, (part + 1) * nA * 64
                nc.sync.dma_start(w_sb[:, c0:c1], w_dram[:, c0:c1])

            kt_tiles = []
            v_tiles = []
            x_tiles = {}
            cs_tiles = {}

            def prefetch(qc):
                t0 = qc * CH
                xa = xtp.tile([128, nA * CH], BF, name=f"x{qc}", tag="x")
                for part in range(2):
                    a0, a1 = part * (nA // 2), (part + 1) * (nA // 2)
                    nc.sync.dma_start(
                        xa[:, a0 * CH:a1 * CH].rearrange(
                            "p (a t) -> p a t", a=nA // 2),
                        xTr[:, a0:a1, t0:t0 + CH])
                x_tiles[qc] = xa
                cos_t = tab.tile([128, CH], BF, name=f"cos{qc}", tag="tab")
                sin_t = tab.tile([128, CH], BF, name=f"sin{qc}", tag="tab")
                nc.sync.dma_start(cos_t[:], cosT[:, t0:t0 + CH])
                nc.sync.dma_start(sin_t[:], sinT[:, t0:t0 + CH])
                cs_tiles[qc] = (cos_t, sin_t)

            # chunk-0 x tiles at per-a granularity, interleaved just-in-time
            # with the weight pieces the q0 chain consumes alongside them.
            xa0 = xtp.tile([128, nA * CH], BF, name="x0", tag="x")

            def xa0_part(a):
                nc.sync.dma_start(xa0[:, a * CH:(a + 1) * CH],
                                  xTr[:, a, 0:CH])

            x_tiles[0] = xa0
            wq_part(0)
            for a in (0, 1, 2, 3):
                xa0_part(a)
            wq_part(1)
            for a in (4, 5, 6, 7):
                xa0_part(a)
            wq_part(2)
            for a in (8, 9, 10, 11):
                xa0_part(a)
            wq_part(3)
            for a in (12, 13, 14, 15):
                xa0_part(a)
            for part in range(2):
                wkv_part(wk_sb, wk2, part)
                wkv_part(wv_sb, wv2, part)
            cos0 = tab.tile([128, CH], BF, name="cos0", tag="tab")
            sin0 = tab.tile([128, CH], BF, name="sin0", tag="tab")
            nc.sync.dma_start(cos0[:], cosT[:, 0:CH])
            nc.sync.dma_start(sin0[:], sinT[:, 0:CH])
            cs_tiles[0] = (cos0, sin0)
            nc.sync.dma_start(mask_sb[:], masks[:])
            nc.sync.dma_start(ones_sb[:], onesM[:])
            for part in range(2):
                c0, c1 = part * D, (part + 1) * D
                nc.sync.dma_start(wo_sb[:, c0:c1], wo2[:, c0:c1])
            prefetch(1)

            def proj_chain(xa, lhs_fn, nm):
                acc = psA.tile([128, CH], FP, name=f"ps_{nm}", tag="acc")
                for a in range(nA):
                    nc.tensor.matmul(
                        acc[:], lhs_fn(a), xa[:, a * CH:(a + 1) * CH],
                        start=(a == 0), stop=(a == nA - 1))
                return acc

            def proj_q0(qc):
                xa = x_tiles[qc]
                q0_ps = proj_chain(
                    xa, lambda a: wq_sb[:, a * 256:a * 256 + 128], f"q0_{qc}")
                q0_sb = work.tile([128, CH], BF, name=f"q0s{qc}", tag="qk",
                                  bufs=4)
                nc.scalar.copy(q0_sb[:], q0_ps[:])
                return q0_sb

            def proj_rest(qc, q0_sb):
                xa = x_tiles.pop(qc)
                cos_t, sin_t = cs_tiles.pop(qc)

                q1_ps = proj_chain(
                    xa, lambda a: wq_sb[:, a * 256 + 128:a * 256 + 256],
                    f"q1_{qc}")
                q1_sb = work.tile([128, CH], BF, name=f"q1s{qc}", tag="qk",
                                  bufs=4)
                nc.scalar.copy(q1_sb[:], q1_ps[:])

                k_ps = proj_chain(
                    xa, lambda a: wk_sb[:, a * 128:(a + 1) * 128], f"k_{qc}")
                k_sb = work.tile([128, CH], BF, name=f"ks{qc}", tag="qk",
                                 bufs=4)
                nc.scalar.copy(k_sb[:], k_ps[:])

                # V transposed (weight-stationary, full-width streams), then
                # de-transpose via the DMA XBAR into natural [t, dh] layout.
                vt_ps = proj_chain(
                    xa, lambda a: wv_sb[:, a * 128:(a + 1) * 128], f"vt_{qc}")
                vt_sb = work.tile([128, CH], BF, name=f"vts{qc}", tag="vt",
                                  bufs=2)
                nc.scalar.copy(vt_sb[:], vt_ps[:])
                vnat = ktv.tile([128, CH], BF, name=f"v{qc}", tag="ktv")
                for s in range(nS):
                    nc.sync.dma_start_transpose(
                        vnat[:, s * 128:(s + 1) * 128],
                        vt_sb[:, s * 128:(s + 1) * 128])
                v_tiles.append(vnat)

                def rope(src_sb, dst, nm):
                    # pair-swap partitions via two strided SBUF->SBUF DMAs
                    sw = work.tile([128, CH], BF, name=f"sw_{nm}", tag="sw",
                                   bufs=4)
                    sr = src_sb[:].rearrange("(i j) t -> i j t", j=2)
                    dr = sw[:].rearrange("(i j) t -> i j t", j=2)
                    # ACT's hwdge queue: keeps rope off the SP queue, which
                    # carries the big x-prefetch transfers.
                    nc.scalar.dma_start(dr[:, 0, :], sr[:, 1, :])
                    nc.scalar.dma_start(dr[:, 1, :], sr[:, 0, :])
                    t1 = work.tile([128, CH], BF, name=f"r1_{nm}", tag="rt",
                                   bufs=4)
                    nc.vector.tensor_mul(t1[:], src_sb[:], cos_t[:])
                    t2 = work.tile([128, CH], BF, name=f"r2_{nm}", tag="rt",
                                   bufs=4)
                    nc.vector.tensor_mul(t2[:], sw[:], sin_t[:])
                    nc.vector.tensor_add(dst, t1[:], t2[:])

                qr0 = work.tile([128, CH], BF, name=f"qr0_{qc}", tag="qr",
                                bufs=4)
                qr1 = work.tile([128, CH], BF, name=f"qr1_{qc}", tag="qr",
                                bufs=4)
                ktt = ktv.tile([128, CH], BF, name=f"kt{qc}", tag="ktv")
                rope(q0_sb, qr0[:], f"q0_{qc}")
                rope(q1_sb, qr1[:], f"q1_{qc}")
                rope(k_sb, ktt[:], f"k_{qc}")
                kt_tiles.append(ktt)
                return qr0, qr1

            wo_flip = [0]

            def wo_items_for(qc, cn0, cn1, tail=False):
                t0 = qc * CH
                items = []
                for s in range(nS):
                    for n in range(nN):
                        def emit(s=s, n=n):
                            w_ps = psA.tile([128, 512], FP,
                                            name=f"w{qc}_{s}_{n}", tag="acc")
                            nc.tensor.matmul(
                                w_ps[:], cn0[:, s * 128:(s + 1) * 128],
                                wo_sb[:, n * 512:(n + 1) * 512],
                                start=True, stop=False)
                            nc.tensor.matmul(
                                w_ps[:], cn1[:, s * 128:(s + 1) * 128],
                                wo_sb[:, D + n * 512:D + (n + 1) * 512],
                                start=False, stop=True)
                            osb = work.tile([128, 512], BF,
                                            name=f"o{qc}_{s}_{n}", tag="osb",
                                            bufs=12)
                            if tail and wo_flip[0] % 2 == 1:
                                nc.scalar.copy(osb[:], w_ps[:])
                            else:
                                nc.vector.tensor_copy(osb[:], w_ps[:])
                            wo_flip[0] += 1
                            nc.sync.dma_start(
                                out[t0 + s * 128:t0 + (s + 1) * 128,
                                    n * 512:(n + 1) * 512], osb[:])
                        items.append(emit)
                return items

            pending_wo = []
            q0_next = proj_q0(0)
            qr = proj_rest(0, q0_next)
            for qc in range(nT):
                qr0, qr1 = qr
                nkp = (qc + 1) * 2          # key-tile pairs per head
                nkt = nkp * 2               # 128-key tiles per head
                cns = []
                accs = {}
                ctxs = {}
                deferred = [None]           # (h, use, p)

                def emit_norm(h):
                    lp = psA.tile([128, CH], FP, name=f"lp{qc}_{h}",
                                  tag="acc")
                    acc = accs[h]
                    nc.tensor.matmul(lp[:], ones_sb[:], acc[:, 0:CH],
                                     start=True, stop=False)
                    nc.tensor.matmul(lp[:], ones_sb[:], acc[:, CH:2 * CH],
                                     start=False, stop=True)
                    ln_t = work.tile([128, CH], FP, name=f"ln{qc}_{h}",
                                     tag="r", bufs=4)
                    nc.scalar.activation(ln_t[:], lp[:],
                                         mybir.ActivationFunctionType.Ln)
                    r_bc = work.tile([128, CH], FP, name=f"r{qc}_{h}",
                                     tag="r", bufs=4)
                    nc.scalar.activation(r_bc[:], ln_t[:],
                                         mybir.ActivationFunctionType.Exp,
                                         scale=-1.0)
                    cn = work.tile([128, CH], BF, name=f"cn{qc}_{h}",
                                   tag="cn", bufs=4)
                    nc.vector.tensor_mul(cn[:], ctxs[h][:], r_bc[:])
                    cns.append(cn)

                def flush_deferred():
                    if deferred[0] is None:
                        return
                    h_, use_, p_ = deferred[0]
                    deferred[0] = None
                    for half in (0, 1):
                        kt = 2 * p_ + half
                        kc, ko = kt // nS, (kt % nS) * 128
                        nc.tensor.matmul(
                            ctxs[h_][:], v_tiles[kc][:, ko:ko + 128],
                            use_[:, half * CH:(half + 1) * CH],
                            start=(kt == 0), stop=(kt == nkt - 1))
                    if p_ == nkp - 1:
                        emit_norm(h_)

                gp = 0                      # global pair counter this chunk
                for h, qrh in enumerate((qr0, qr1)):
                    accs[h] = accp.tile([128, 2 * CH], BF,
                                        name=f"acc{qc}_{h}", tag="accp")
                    ctxs[h] = psC.tile([128, CH], FP, name=f"ctx{qc}_{h}",
                                       tag="ctx")
                    for p in range(nkp):
                        Sp = psS.tile([128, 2 * CH], FP,
                                      name=f"S{qc}_{h}_{p}", tag="s")
                        for half in (0, 1):
                            kt = 2 * p + half
                            kc, ko = kt // nS, (kt % nS) * 128
                            nc.tensor.matmul(
                                Sp[:, half * CH:(half + 1) * CH],
                                kt_tiles[kc][:, ko:ko + 128], qrh[:],
                                start=True, stop=True)
                        Pp = pp.tile([128, 2 * CH], BF,
                                     name=f"P{qc}_{h}_{p}", tag="p")
                        nc.scalar.activation(
                            Pp[:], Sp[:], mybir.ActivationFunctionType.Exp,
                            scale=ISQ)
                        dp = p - 2 * qc
                        if dp >= 0:
                            Pm = pp.tile([128, 2 * CH], BF,
                                         name=f"Pm{qc}_{h}_{p}", tag="p")
                            nc.vector.tensor_mul(
                                Pm[:], Pp[:],
                                mask_sb[:, dp * 2 * CH:(dp + 1) * 2 * CH])
                            use = Pm
                        else:
                            use = Pp
                        if p == 0:
                            nc.vector.tensor_copy(accs[h][:], use[:])
                        else:
                            nc.vector.tensor_add(accs[h][:], accs[h][:],
                                                 use[:])
                        flush_deferred()
                        deferred[0] = (h, use, p)
                        if pending_wo and (gp % 2 == 1):
                            pending_wo.pop(0)()
                        gp += 1

                # chunk epilogue: start next chunk's first projection chain
                # before the final deferred ctx + normalization, so the PE
                # stream never drains behind ACT/DVE.
                if qc + 1 < nT:
                    q0_next = proj_q0(qc + 1)
                flush_deferred()
                while pending_wo:
                    pending_wo.pop(0)()
                pending_wo = wo_items_for(qc, cns[0], cns[1],
                                          tail=(qc == nT - 1))
                if qc + 1 < nT:
                    qr = proj_rest(qc + 1, q0_next)
                if qc + 2 < nT:
                    prefetch(qc + 2)
            while pending_wo:
                pending_wo.pop(0)()

    _split_multi_waits(nc, 1)
    return nc


# --------------------------------------------------------------------------
def host_prep(x, WQ, WK, WV, WO):
    ROPE_BASE = 10000.0

    xTc = np.ascontiguousarray(
        np.asarray(x, dtype=np.float32).reshape(T, D).T).astype(BFNP)

    omega = 1.0 / (ROPE_BASE ** (np.arange(0, DH, 2, dtype=np.float64) / DH))
    ang = np.outer(omega, np.arange(T, dtype=np.float64))
    cosT = np.repeat(np.cos(ang), 2, axis=0).astype(BFNP)
    sgn = np.tile(np.array([-1.0, 1.0]), DH // 2)[:, None]
    sinT = (np.repeat(np.sin(ang), 2, axis=0) * sgn).astype(BFNP)

    p_i = np.arange(128)[:, None]
    f_i = np.arange(CH)[None, :]
    masks = np.concatenate(
        [(128 * dl + p_i <= f_i).astype(np.float32) for dl in range(nS)],
        axis=1).astype(BFNP)

    def tile_pmaj(w, ncols):
        return np.ascontiguousarray(
            np.asarray(w, dtype=np.float32).reshape(nA, 128, ncols)
            .transpose(1, 0, 2).reshape(128, nA * ncols)).astype(BFNP)

    in_maps = []
    for c in range(N_CORES):
        kv = c // 2
        wo_c = np.asarray(WO, dtype=np.float32)[256 * c:256 * (c + 1), :]
        in_maps.append({
            "xT": xTc,
            "wq2": tile_pmaj(np.asarray(WQ)[:, 256 * c:256 * (c + 1)], 256),
            "wk2": tile_pmaj(np.asarray(WK)[:, 128 * kv:128 * (kv + 1)], 128),
            "wv2": tile_pmaj(np.asarray(WV)[:, 128 * kv:128 * (kv + 1)], 128),
            "wo2": np.ascontiguousarray(
                wo_c.reshape(2, 128, D).transpose(1, 0, 2)
                .reshape(128, 2 * D)).astype(BFNP),
            "cosT": cosT, "sinT": sinT, "masks": masks,
            "onesM": np.ones((128, 128), dtype=BFNP),
        })
    return in_maps


_NC_CACHE = {}


def _get_nc():
    if "nc" not in _NC_CACHE:
        _NC_CACHE["nc"] = build_nc()
    return _NC_CACHE["nc"]


def run_on_hw(inputs, trace=False):
    """Returns (out [1,T,D] fp32, BassKernelResults)."""
    nc = _get_nc()
    in_maps = host_prep(inputs["x"], inputs["WQ"], inputs["WK"],
                        inputs["WV"], inputs["WO"])
    res = run_bass_kernel_spmd(nc, in_maps, list(range(N_CORES)),
                               trace=trace)
    acc = np.zeros((T, D), dtype=np.float32)
    for c in range(N_CORES):
        acc += res.results[c]["out"].astype(np.float32)
    return acc[None], res


def kernel(x, WQ, WK, WV, WO):
    out, _ = run_on_hw({"x": x, "WQ": WQ, "WK": WK, "WV": WV, "WO": WO})
    return out


# revision 15
# speedup vs baseline: 1.0177x; 1.0177x over previous
"""GroupedQueryAttention (B=1, T=4096, D=2048, 16 heads / 4 kv heads, RoPE,
causal) on 8 Trainium2 NeuronCores.

Sharding: head tensor-parallel. Core c owns q-heads {2c, 2c+1} and kv head
c//2 (WQ/WO split along head dim, WK/WV along kv-head dim). Each core
computes its partial out = ctx_heads @ WO_slice over the full sequence;
partials are summed across cores on the host gather side.

v3 pipeline:
 - softmax denominator: per-pair DVE accumulation of P into a bf16 running
   sum + one ones-matrix matmul per (head, chunk) that also broadcasts the
   sums across partitions; reciprocal via ACT exp(-Ln(l)) (same table set
   as the softmax Exp, so no ACT table switches).
 - exp batched over key-tile PAIRS: one ACT instruction per [128,1024]
   PSUM S-pair (amortizes the 352-cycle ACT fixed overhead).
 - V produced transposed (weight-stationary chains, N=512 streams) then
   de-transposed with dma_start_transpose (XBAR).
 - flat cross-head pair stream with one-pair-deferred ctx emission and
   deferred per-head normalization, so the PE never drains at head or
   chunk boundaries (keeps HAM at full clock).
 - WO matmuls of chunk qc interleaved into chunk qc+1's attention stream;
   PSUM->SBUF output copies alternate between ACT and DVE.
 - split weight/x DMAs + 2-chunk-ahead prefetch to cut startup and SP
   queue head-of-line blocking; bf16 rope; bf16 partial-output DMA.
"""

import math

import numpy as np
import ml_dtypes

import concourse.bass as bass
import concourse.mybir as mybir
import concourse.tile as tile
from concourse.bass_utils import run_bass_kernel_spmd

FP = mybir.dt.float32
BF = mybir.dt.bfloat16
BFNP = ml_dtypes.bfloat16

T, D, DH = 4096, 2048, 128      # seq len, model dim, head dim
CH = 512                        # query-chunk (free dim of attention matmuls)
N_CORES = 8

nT = T // CH          # 8 q-chunks
nA = D // 128         # 16 contraction tiles
nS = CH // 128        # 4 128-subtiles per chunk
nN = D // 512         # 4 output column tiles
ISQ = 1.0 / math.sqrt(float(DH))


# --------------------------------------------------------------------------
# workaround: this walrus build rejects instructions carrying >1 sem-waits
# (setupSyncWait "Too many sync wait commands"); split extras into NoOps.
_WS_CTR = [0]


def _split_multi_waits(nc, limit=1):
    for f in nc.m.functions:
        for bb in f.blocks:
            il = bb.instructions
            i = 0
            while i < len(il):
                inst = il[i]
                si = getattr(inst, "sync_info", None)
                if si is not None and len(si.on_wait) > limit:
                    waits = list(si.on_wait)
                    keep, rest = waits[:limit], waits[limit:]
                    nops = []
                    for j in range(0, len(rest), limit):
                        _WS_CTR[0] += 1
                        n = mybir.InstNoOp(name=f"waitsplit-{_WS_CTR[0]}")
                        n.engine = inst.engine
                        n.sync_info = mybir.SyncInfo(
                            on_wait=rest[j:j + limit], on_update=[])
                        nops.append(n)
                    inst.sync_info = mybir.SyncInfo(
                        on_wait=keep, on_update=list(si.on_update))
                    for k, n in enumerate(nops):
                        il.insert(i + k, n)
                    i += len(nops)
                i += 1


# --------------------------------------------------------------------------
def build_nc():
    nc = bass.Bass()

    xT = nc.dram_tensor("xT", [D, T], BF, kind="ExternalInput")
    wq2 = nc.dram_tensor("wq2", [128, nA * 256], BF, kind="ExternalInput")
    wk2 = nc.dram_tensor("wk2", [128, nA * 128], BF, kind="ExternalInput")
    wv2 = nc.dram_tensor("wv2", [128, nA * 128], BF, kind="ExternalInput")
    wo2 = nc.dram_tensor("wo2", [128, 2 * D], BF, kind="ExternalInput")
    cosT = nc.dram_tensor("cosT", [128, T], BF, kind="ExternalInput")
    sinT = nc.dram_tensor("sinT", [128, T], BF, kind="ExternalInput")
    masks = nc.dram_tensor("masks", [128, nS * CH], BF, kind="ExternalInput")
    onesM = nc.dram_tensor("onesM", [128, 128], BF, kind="ExternalInput")
    out = nc.dram_tensor("out", [T, D], BF, kind="ExternalOutput")

    xTr = xT.rearrange("(a p) t -> p a t", p=128)

    with tile.TileContext(nc) as tc:
        with (
            tc.tile_pool(name="res", bufs=1) as res,
            tc.tile_pool(name="ktv", bufs=2 * nT) as ktv,
            tc.tile_pool(name="xt", bufs=3) as xtp,
            tc.tile_pool(name="tab", bufs=6) as tab,
            tc.tile_pool(name="work", bufs=2) as work,
            tc.tile_pool(name="pp", bufs=6) as pp,
            tc.tile_pool(name="accp", bufs=2) as accp,
            tc.tile_pool(name="psS", bufs=2, space="PSUM") as psS,
            tc.tile_pool(name="psC", bufs=2, space="PSUM") as psC,
            tc.tile_pool(name="psA", bufs=2, space="PSUM") as psA,
        ):
            wq_sb = res.tile([128, nA * 256], BF, name="wq_sb")
            wk_sb = res.tile([128, nA * 128], BF, name="wk_sb")
            wv_sb = res.tile([128, nA * 128], BF, name="wv_sb")
            wo_sb = res.tile([128, 2 * D], BF, name="wo_sb")
            mask_sb = res.tile([128, nS * CH], BF, name="mask_sb")
            ones_sb = res.tile([128, 128], BF, name="ones_sb")

            def wq_part(part):
                c0, c1 = part * nA * 64, (part + 1) * nA * 64
                nc.sync.dma_start(wq_sb[:, c0:c1], wq2[:, c0:c1])

            def wkv_part(w_sb, w_dram, part):
                c0, c1 = part * nA * 64, (part + 1) * nA * 64
                nc.sync.dma_start(w_sb[:, c0:c1], w_dram[:, c0:c1])

            kt_tiles = []
            v_tiles = []
            x_tiles = {}
            cs_tiles = {}

            def prefetch(qc):
                t0 = qc * CH
                xa = xtp.tile([128, nA * CH], BF, name=f"x{qc}", tag="x")
                for part in range(2):
                    a0, a1 = part * (nA // 2), (part + 1) * (nA // 2)
                    nc.sync.dma_start(
                        xa[:, a0 * CH:a1 * CH].rearrange(
                            "p (a t) -> p a t", a=nA // 2),
                        xTr[:, a0:a1, t0:t0 + CH])
                x_tiles[qc] = xa
                cos_t = tab.tile([128, CH], BF, name=f"cos{qc}", tag="tab")
                sin_t = tab.tile([128, CH], BF, name=f"sin{qc}", tag="tab")
                nc.sync.dma_start(cos_t[:], cosT[:, t0:t0 + CH])
                nc.sync.dma_start(sin_t[:], sinT[:, t0:t0 + CH])
                cs_tiles[qc] = (cos_t, sin_t)

            # chunk-0 x tiles at per-a granularity, interleaved just-in-time
            # with the weight pieces the q0 chain consumes alongside them.
            xa0 = xtp.tile([128, nA * CH], BF, name="x0", tag="x")

            def xa0_part(a):
                nc.sync.dma_start(xa0[:, a * CH:(a + 1) * CH],
                                  xTr[:, a, 0:CH])

            x_tiles[0] = xa0
            wq_part(0)
            for a in (0, 1, 2, 3):
                xa0_part(a)
            wq_part(1)
            for a in (4, 5, 6, 7):
                xa0_part(a)
            wq_part(2)
            for a in (8, 9, 10, 11):
                xa0_part(a)
            wq_part(3)
            for a in (12, 13, 14, 15):
                xa0_part(a)
            for part in range(2):
                wkv_part(wk_sb, wk2, part)
                wkv_part(wv_sb, wv2, part)
            cos0 = tab.tile([128, CH], BF, name="cos0", tag="tab")
            sin0 = tab.tile([128, CH], BF, name="sin0", tag="tab")
            nc.sync.dma_start(cos0[:], cosT[:, 0:CH])
            nc.sync.dma_start(sin0[:], sinT[:, 0:CH])
            cs_tiles[0] = (cos0, sin0)
            nc.sync.dma_start(mask_sb[:], masks[:])
            nc.sync.dma_start(ones_sb[:], onesM[:])
            for part in range(2):
                c0, c1 = part * D, (part + 1) * D
                nc.sync.dma_start(wo_sb[:, c0:c1], wo2[:, c0:c1])
            prefetch(1)

            def proj_chain(xa, lhs_fn, nm):
                acc = psA.tile([128, CH], FP, name=f"ps_{nm}", tag="acc")
                for a in range(nA):
                    nc.tensor.matmul(
                        acc[:], lhs_fn(a), xa[:, a * CH:(a + 1) * CH],
                        start=(a == 0), stop=(a == nA - 1))
                return acc

            def proj_q0(qc):
                xa = x_tiles[qc]
                q0_ps = proj_chain(
                    xa, lambda a: wq_sb[:, a * 256:a * 256 + 128], f"q0_{qc}")
                q0_sb = work.tile([128, CH], BF, name=f"q0s{qc}", tag="qk",
                                  bufs=4)
                nc.scalar.copy(q0_sb[:], q0_ps[:])
                return q0_sb

            def proj_rest(qc, q0_sb):
                xa = x_tiles.pop(qc)
                cos_t, sin_t = cs_tiles.pop(qc)

                q1_ps = proj_chain(
                    xa, lambda a: wq_sb[:, a * 256 + 128:a * 256 + 256],
                    f"q1_{qc}")
                q1_sb = work.tile([128, CH], BF, name=f"q1s{qc}", tag="qk",
                                  bufs=4)
                nc.scalar.copy(q1_sb[:], q1_ps[:])

                k_ps = proj_chain(
                    xa, lambda a: wk_sb[:, a * 128:(a + 1) * 128], f"k_{qc}")
                k_sb = work.tile([128, CH], BF, name=f"ks{qc}", tag="qk",
                                 bufs=4)
                nc.scalar.copy(k_sb[:], k_ps[:])

                # V transposed (weight-stationary, full-width streams), then
                # de-transpose via the DMA XBAR into natural [t, dh] layout.
                vt_ps = proj_chain(
                    xa, lambda a: wv_sb[:, a * 128:(a + 1) * 128], f"vt_{qc}")
                vt_sb = work.tile([128, CH], BF, name=f"vts{qc}", tag="vt",
                                  bufs=2)
                nc.scalar.copy(vt_sb[:], vt_ps[:])
                vnat = ktv.tile([128, CH], BF, name=f"v{qc}", tag="ktv")
                for s in range(nS):
                    nc.sync.dma_start_transpose(
                        vnat[:, s * 128:(s + 1) * 128],
                        vt_sb[:, s * 128:(s + 1) * 128])
                v_tiles.append(vnat)

                def rope(src_sb, dst, nm):
                    # pair-swap partitions via two strided SBUF->SBUF DMAs
                    sw = work.tile([128, CH], BF, name=f"sw_{nm}", tag="sw",
                                   bufs=4)
                    sr = src_sb[:].rearrange("(i j) t -> i j t", j=2)
                    dr = sw[:].rearrange("(i j) t -> i j t", j=2)
                    # ACT's hwdge queue: keeps rope off the SP queue, which
                    # carries the big x-prefetch transfers.
                    nc.scalar.dma_start(dr[:, 0, :], sr[:, 1, :])
                    nc.scalar.dma_start(dr[:, 1, :], sr[:, 0, :])
                    t1 = work.tile([128, CH], BF, name=f"r1_{nm}", tag="rt",
                                   bufs=4)
                    nc.vector.tensor_mul(t1[:], src_sb[:], cos_t[:])
                    t2 = work.tile([128, CH], BF, name=f"r2_{nm}", tag="rt",
                                   bufs=4)
                    nc.vector.tensor_mul(t2[:], sw[:], sin_t[:])
                    nc.vector.tensor_add(dst, t1[:], t2[:])

                qr0 = work.tile([128, CH], BF, name=f"qr0_{qc}", tag="qr",
                                bufs=4)
                qr1 = work.tile([128, CH], BF, name=f"qr1_{qc}", tag="qr",
                                bufs=4)
                ktt = ktv.tile([128, CH], BF, name=f"kt{qc}", tag="ktv")
                rope(q0_sb, qr0[:], f"q0_{qc}")
                rope(q1_sb, qr1[:], f"q1_{qc}")
                rope(k_sb, ktt[:], f"k_{qc}")
                kt_tiles.append(ktt)
                return qr0, qr1

            wo_flip = [0]

            def wo_items_for(qc, cn0, cn1, tail=False):
                t0 = qc * CH
                items = []
                for s in range(nS):
                    for n in range(nN):
                        def emit(s=s, n=n):
                            w_ps = psA.tile([128, 512], FP,
                                            name=f"w{qc}_{s}_{n}", tag="acc")
                            nc.tensor.matmul(
                                w_ps[:], cn0[:, s * 128:(s + 1) * 128],
                                wo_sb[:, n * 512:(n + 1) * 512],
                                start=True, stop=False)
                            nc.tensor.matmul(
                                w_ps[:], cn1[:, s * 128:(s + 1) * 128],
                                wo_sb[:, D + n * 512:D + (n + 1) * 512],
                                start=False, stop=True)
                            osb = work.tile([128, 512], BF,
                                            name=f"o{qc}_{s}_{n}", tag="osb",
                                            bufs=12)
                            if tail and wo_flip[0] % 2 == 1:
                                nc.scalar.copy(osb[:], w_ps[:])
                            else:
                                nc.vector.tensor_copy(osb[:], w_ps[:])
                            wo_flip[0] += 1
                            nc.sync.dma_start(
                                out[t0 + s * 128:t0 + (s + 1) * 128,
                                    n * 512:(n + 1) * 512], osb[:])
                        items.append(emit)
                return items

            pending_wo = []
            q0_next = proj_q0(0)
            qr = proj_rest(0, q0_next)
            for qc in range(nT):
                if qc + 2 < nT:
                    prefetch(qc + 2)
                qr0, qr1 = qr
                nkp = (qc + 1) * 2          # key-tile pairs per head
                nkt = nkp * 2               # 128-key tiles per head
                cns = []
                accs = {}
                ctxs = {}
                deferred = [None]           # (h, use, p)

                def emit_norm(h):
                    lp = psA.tile([128, CH], FP, name=f"lp{qc}_{h}",
                                  tag="acc")
                    acc = accs[h]
                    nc.tensor.matmul(lp[:], ones_sb[:], acc[:, 0:CH],
                                     start=True, stop=False)
                    nc.tensor.matmul(lp[:], ones_sb[:], acc[:, CH:2 * CH],
                                     start=False, stop=True)
                    ln_t = work.tile([128, CH], FP, name=f"ln{qc}_{h}",
                                     tag="r", bufs=4)
                    nc.scalar.activation(ln_t[:], lp[:],
                                         mybir.ActivationFunctionType.Ln)
                    r_bc = work.tile([128, CH], FP, name=f"r{qc}_{h}",
                                     tag="r", bufs=4)
                    nc.scalar.activation(r_bc[:], ln_t[:],
                                         mybir.ActivationFunctionType.Exp,
                                         scale=-1.0)
                    cn = work.tile([128, CH], BF, name=f"cn{qc}_{h}",
                                   tag="cn", bufs=4)
                    nc.vector.tensor_mul(cn[:], ctxs[h][:], r_bc[:])
                    cns.append(cn)

                def flush_deferred():
                    if deferred[0] is None:
                        return
                    h_, use_, p_ = deferred[0]
                    deferred[0] = None
                    for half in (0, 1):
                        kt = 2 * p_ + half
                        kc, ko = kt // nS, (kt % nS) * 128
                        nc.tensor.matmul(
                            ctxs[h_][:], v_tiles[kc][:, ko:ko + 128],
                            use_[:, half * CH:(half + 1) * CH],
                            start=(kt == 0), stop=(kt == nkt - 1))
                    if p_ == nkp - 1:
                        emit_norm(h_)

                gp = 0                      # global pair counter this chunk
                for h, qrh in enumerate((qr0, qr1)):
                    accs[h] = accp.tile([128, 2 * CH], BF,
                                        name=f"acc{qc}_{h}", tag="accp")
                    ctxs[h] = psC.tile([128, CH], FP, name=f"ctx{qc}_{h}",
                                       tag="ctx")
                    for p in range(nkp):
                        Sp = psS.tile([128, 2 * CH], FP,
                                      name=f"S{qc}_{h}_{p}", tag="s")
                        for half in (0, 1):
                            kt = 2 * p + half
                            kc, ko = kt // nS, (kt % nS) * 128
                            nc.tensor.matmul(
                                Sp[:, half * CH:(half + 1) * CH],
                                kt_tiles[kc][:, ko:ko + 128], qrh[:],
                                start=True, stop=True)
                        Pp = pp.tile([128, 2 * CH], BF,
                                     name=f"P{qc}_{h}_{p}", tag="p")
                        nc.scalar.activation(
                            Pp[:], Sp[:], mybir.ActivationFunctionType.Exp,
                            scale=ISQ)
                        dp = p - 2 * qc
                        if dp >= 0:
                            Pm = pp.tile([128, 2 * CH], BF,
                                         name=f"Pm{qc}_{h}_{p}", tag="p")
                            nc.vector.tensor_mul(
                                Pm[:], Pp[:],
                                mask_sb[:, dp * 2 * CH:(dp + 1) * 2 * CH])
                            use = Pm
                        else:
                            use = Pp
                        if p == 0:
                            nc.vector.tensor_copy(accs[h][:], use[:])
                        else:
                            nc.vector.tensor_add(accs[h][:], accs[h][:],
                                                 use[:])
                        flush_deferred()
                        deferred[0] = (h, use, p)
                        if pending_wo and (gp % 2 == 1):
                            pending_wo.pop(0)()
                        gp += 1

                # chunk epilogue: start next chunk's first projection chain
                # before the final deferred ctx + normalization, so the PE
                # stream never drains behind ACT/DVE.
                if qc + 1 < nT:
                    q0_next = proj_q0(qc + 1)
                flush_deferred()
                while pending_wo:
                    pending_wo.pop(0)()
                pending_wo = wo_items_for(qc, cns[0], cns[1],
                                          tail=(qc == nT - 1))
                if qc + 1 < nT:
                    qr = proj_rest(qc + 1, q0_next)
            while pending_wo:
                pending_wo.pop(0)()

    _split_multi_waits(nc, 1)
    return nc


# --------------------------------------------------------------------------
def host_prep(x, WQ, WK, WV, WO):
    ROPE_BASE = 10000.0

    xTc = np.ascontiguousarray(
        np.asarray(x, dtype=np.float32).reshape(T, D).T).astype(BFNP)

    omega = 1.0 / (ROPE_BASE ** (np.arange(0, DH, 2, dtype=np.float64) / DH))
    ang = np.outer(omega, np.arange(T, dtype=np.float64))
    cosT = np.repeat(np.cos(ang), 2, axis=0).astype(BFNP)
    sgn = np.tile(np.array([-1.0, 1.0]), DH // 2)[:, None]
    sinT = (np.repeat(np.sin(ang), 2, axis=0) * sgn).astype(BFNP)

    p_i = np.arange(128)[:, None]
    f_i = np.arange(CH)[None, :]
    masks = np.concatenate(
        [(128 * dl + p_i <= f_i).astype(np.float32) for dl in range(nS)],
        axis=1).astype(BFNP)

    def tile_pmaj(w, ncols):
        return np.ascontiguousarray(
            np.asarray(w, dtype=np.float32).reshape(nA, 128, ncols)
            .transpose(1, 0, 2).reshape(128, nA * ncols)).astype(BFNP)

    in_maps = []
    for c in range(N_CORES):
        kv = c // 2
        wo_c = np.asarray(WO, dtype=np.float32)[256 * c:256 * (c + 1), :]
        in_maps.append({
            "xT": xTc,
            "wq2": tile_pmaj(np.asarray(WQ)[:, 256 * c:256 * (c + 1)], 256),
            "wk2": tile_pmaj(np.asarray(WK)[:, 128 * kv:128 * (kv + 1)], 128),
            "wv2": tile_pmaj(np.asarray(WV)[:, 128 * kv:128 * (kv + 1)], 128),
            "wo2": np.ascontiguousarray(
                wo_c.reshape(2, 128, D).transpose(1, 0, 2)
                .reshape(128, 2 * D)).astype(BFNP),
            "cosT": cosT, "sinT": sinT, "masks": masks,
            "onesM": np.ones((128, 128), dtype=BFNP),
        })
    return in_maps


_NC_CACHE = {}


def _get_nc():
    if "nc" not in _NC_CACHE:
        _NC_CACHE["nc"] = build_nc()
    return _NC_CACHE["nc"]


def run_on_hw(inputs, trace=False):
    """Returns (out [1,T,D] fp32, BassKernelResults)."""
    nc = _get_nc()
    in_maps = host_prep(inputs["x"], inputs["WQ"], inputs["WK"],
                        inputs["WV"], inputs["WO"])
    res = run_bass_kernel_spmd(nc, in_maps, list(range(N_CORES)),
                               trace=trace)
    acc = np.zeros((T, D), dtype=np.float32)
    for c in range(N_CORES):
        acc += res.results[c]["out"].astype(np.float32)
    return acc[None], res


def kernel(x, WQ, WK, WV, WO):
    out, _ = run_on_hw({"x": x, "WQ": WQ, "WK": WK, "WV": WV, "WO": WO})
    return out


# revision 18
# speedup vs baseline: 1.0386x; 1.0206x over previous
"""GroupedQueryAttention (B=1, T=4096, D=2048, 16 heads / 4 kv heads, RoPE,
causal) on 8 Trainium2 NeuronCores.

Sharding: head tensor-parallel. Core c owns q-heads {2c, 2c+1} and kv head
c//2 (WQ/WO split along head dim, WK/WV along kv-head dim). Each core
computes its partial out = ctx_heads @ WO_slice over the full sequence;
partials are summed across cores on the host gather side.

v3 pipeline:
 - softmax denominator: per-pair DVE accumulation of P into a bf16 running
   sum + one ones-matrix matmul per (head, chunk) that also broadcasts the
   sums across partitions; reciprocal via ACT exp(-Ln(l)) (same table set
   as the softmax Exp, so no ACT table switches).
 - exp batched over key-tile PAIRS: one ACT instruction per [128,1024]
   PSUM S-pair (amortizes the 352-cycle ACT fixed overhead).
 - V produced transposed (weight-stationary chains, N=512 streams) then
   de-transposed with dma_start_transpose (XBAR).
 - flat cross-head pair stream with one-pair-deferred ctx emission and
   deferred per-head normalization, so the PE never drains at head or
   chunk boundaries (keeps HAM at full clock).
 - WO matmuls of chunk qc interleaved into chunk qc+1's attention stream;
   PSUM->SBUF output copies alternate between ACT and DVE.
 - split weight/x DMAs + 2-chunk-ahead prefetch to cut startup and SP
   queue head-of-line blocking; bf16 rope; bf16 partial-output DMA.
"""

import math

import numpy as np
import ml_dtypes

import concourse.bass as bass
import concourse.mybir as mybir
import concourse.tile as tile
from concourse.bass_utils import run_bass_kernel_spmd

FP = mybir.dt.float32
BF = mybir.dt.bfloat16
BFNP = ml_dtypes.bfloat16

T, D, DH = 4096, 2048, 128      # seq len, model dim, head dim
CH = 512                        # query-chunk (free dim of attention matmuls)
N_CORES = 8

nT = T // CH          # 8 q-chunks
nA = D // 128         # 16 contraction tiles
nS = CH // 128        # 4 128-subtiles per chunk
nN = D // 512         # 4 output column tiles
ISQ = 1.0 / math.sqrt(float(DH))


# --------------------------------------------------------------------------
# workaround: this walrus build rejects instructions carrying >1 sem-waits
# (setupSyncWait "Too many sync wait commands"); split extras into NoOps.
_WS_CTR = [0]


def _split_multi_waits(nc, limit=1):
    for f in nc.m.functions:
        for bb in f.blocks:
            il = bb.instructions
            i = 0
            while i < len(il):
                inst = il[i]
                si = getattr(inst, "sync_info", None)
                if si is not None and len(si.on_wait) > limit:
                    waits = list(si.on_wait)
                    keep, rest = waits[:limit], waits[limit:]
                    nops = []
                    for j in range(0, len(rest), limit):
                        _WS_CTR[0] += 1
                        n = mybir.InstNoOp(name=f"waitsplit-{_WS_CTR[0]}")
                        n.engine = inst.engine
                        n.sync_info = mybir.SyncInfo(
                            on_wait=rest[j:j + limit], on_update=[])
                        nops.append(n)
                    inst.sync_info = mybir.SyncInfo(
                        on_wait=keep, on_update=list(si.on_update))
                    for k, n in enumerate(nops):
                        il.insert(i + k, n)
                    i += len(nops)
                i += 1


# --------------------------------------------------------------------------
def build_nc():
    nc = bass.Bass()

    xT = nc.dram_tensor("xT", [D, T], BF, kind="ExternalInput")
    wq2 = nc.dram_tensor("wq2", [128, nA * 256], BF, kind="ExternalInput")
    wk2 = nc.dram_tensor("wk2", [128, nA * 128], BF, kind="ExternalInput")
    wv2 = nc.dram_tensor("wv2", [128, nA * 128], BF, kind="ExternalInput")
    wo2 = nc.dram_tensor("wo2", [128, 2 * D], BF, kind="ExternalInput")
    cosT = nc.dram_tensor("cosT", [128, T], BF, kind="ExternalInput")
    sinT = nc.dram_tensor("sinT", [128, T], BF, kind="ExternalInput")
    masks = nc.dram_tensor("masks", [128, nS * CH], BF, kind="ExternalInput")
    masksP = nc.dram_tensor("masksP", [128, 1280], BF, kind="ExternalInput")
    onesM = nc.dram_tensor("onesM", [128, 128], BF, kind="ExternalInput")
    out = nc.dram_tensor("out", [T, D], BF, kind="ExternalOutput")

    xTr = xT.rearrange("(a p) t -> p a t", p=128)

    with tile.TileContext(nc) as tc:
        with (
            tc.tile_pool(name="res", bufs=1) as res,
            tc.tile_pool(name="ktv", bufs=2 * nT) as ktv,
            tc.tile_pool(name="xt", bufs=3) as xtp,
            tc.tile_pool(name="tab", bufs=6) as tab,
            tc.tile_pool(name="work", bufs=2) as work,
            tc.tile_pool(name="pp", bufs=6) as pp,
            tc.tile_pool(name="accp", bufs=2) as accp,
            tc.tile_pool(name="psS", bufs=2, space="PSUM") as psS,
            tc.tile_pool(name="psC", bufs=2, space="PSUM") as psC,
            tc.tile_pool(name="psA", bufs=2, space="PSUM") as psA,
        ):
            wq_sb = res.tile([128, nA * 256], BF, name="wq_sb")
            wk_sb = res.tile([128, nA * 128], BF, name="wk_sb")
            wv_sb = res.tile([128, nA * 128], BF, name="wv_sb")
            wo_sb = res.tile([128, 2 * D], BF, name="wo_sb")
            mask_sb = res.tile([128, nS * CH], BF, name="mask_sb")
            maskP_sb = res.tile([128, 1280], BF, name="maskP_sb")
            ones_sb = res.tile([128, 128], BF, name="ones_sb")

            def wq_part(part):
                c0, c1 = part * nA * 64, (part + 1) * nA * 64
                nc.sync.dma_start(wq_sb[:, c0:c1], wq2[:, c0:c1])

            def wkv_part(w_sb, w_dram, part):
                c0, c1 = part * nA * 64, (part + 1) * nA * 64
                nc.sync.dma_start(w_sb[:, c0:c1], w_dram[:, c0:c1])

            kt_tiles = []
            v_tiles = []
            x_tiles = {}
            cs_tiles = {}

            def prefetch(qc):
                t0 = qc * CH
                xa = xtp.tile([128, nA * CH], BF, name=f"x{qc}", tag="x")
                for part in range(2):
                    a0, a1 = part * (nA // 2), (part + 1) * (nA // 2)
                    nc.sync.dma_start(
                        xa[:, a0 * CH:a1 * CH].rearrange(
                            "p (a t) -> p a t", a=nA // 2),
                        xTr[:, a0:a1, t0:t0 + CH])
                x_tiles[qc] = xa
                cos_t = tab.tile([128, CH], BF, name=f"cos{qc}", tag="tab")
                sin_t = tab.tile([128, CH], BF, name=f"sin{qc}", tag="tab")
                nc.sync.dma_start(cos_t[:], cosT[:, t0:t0 + CH])
                nc.sync.dma_start(sin_t[:], sinT[:, t0:t0 + CH])
                cs_tiles[qc] = (cos_t, sin_t)

            # chunk-0 x tiles at per-a granularity, interleaved just-in-time
            # with the weight pieces the q0 chain consumes alongside them.
            xa0 = xtp.tile([128, nA * CH], BF, name="x0", tag="x")

            def xa0_part(a):
                nc.sync.dma_start(xa0[:, a * CH:(a + 1) * CH],
                                  xTr[:, a, 0:CH])

            x_tiles[0] = xa0
            wq_part(0)
            for a in (0, 1, 2, 3):
                xa0_part(a)
            wq_part(1)
            for a in (4, 5, 6, 7):
                xa0_part(a)
            wq_part(2)
            for a in (8, 9, 10, 11):
                xa0_part(a)
            wq_part(3)
            for a in (12, 13, 14, 15):
                xa0_part(a)
            for part in range(2):
                wkv_part(wk_sb, wk2, part)
                wkv_part(wv_sb, wv2, part)
            cos0 = tab.tile([128, CH], BF, name="cos0", tag="tab")
            sin0 = tab.tile([128, CH], BF, name="sin0", tag="tab")
            nc.sync.dma_start(cos0[:], cosT[:, 0:CH])
            nc.sync.dma_start(sin0[:], sinT[:, 0:CH])
            cs_tiles[0] = (cos0, sin0)
            nc.sync.dma_start(mask_sb[:], masks[:])
            nc.sync.dma_start(maskP_sb[:], masksP[:])
            nc.sync.dma_start(ones_sb[:], onesM[:])
            for part in range(2):
                c0, c1 = part * D, (part + 1) * D
                nc.sync.dma_start(wo_sb[:, c0:c1], wo2[:, c0:c1])
            prefetch(1)

            def proj_chain(xa, lhs_fn, nm):
                acc = psA.tile([128, CH], FP, name=f"ps_{nm}", tag="acc")
                for a in range(nA):
                    nc.tensor.matmul(
                        acc[:], lhs_fn(a), xa[:, a * CH:(a + 1) * CH],
                        start=(a == 0), stop=(a == nA - 1))
                return acc

            def proj_q0(qc):
                xa = x_tiles[qc]
                q0_ps = proj_chain(
                    xa, lambda a: wq_sb[:, a * 256:a * 256 + 128], f"q0_{qc}")
                q0_sb = work.tile([128, CH], BF, name=f"q0s{qc}", tag="qk",
                                  bufs=4)
                nc.scalar.copy(q0_sb[:], q0_ps[:])
                return q0_sb

            def proj_rest(qc, q0_sb):
                xa = x_tiles.pop(qc)
                cos_t, sin_t = cs_tiles.pop(qc)

                q1_ps = proj_chain(
                    xa, lambda a: wq_sb[:, a * 256 + 128:a * 256 + 256],
                    f"q1_{qc}")
                q1_sb = work.tile([128, CH], BF, name=f"q1s{qc}", tag="qk",
                                  bufs=4)
                nc.scalar.copy(q1_sb[:], q1_ps[:])

                k_ps = proj_chain(
                    xa, lambda a: wk_sb[:, a * 128:(a + 1) * 128], f"k_{qc}")
                k_sb = work.tile([128, CH], BF, name=f"ks{qc}", tag="qk",
                                 bufs=4)
                nc.scalar.copy(k_sb[:], k_ps[:])

                # V transposed (weight-stationary, full-width streams), then
                # de-transpose via the DMA XBAR into natural [t, dh] layout.
                vt_ps = proj_chain(
                    xa, lambda a: wv_sb[:, a * 128:(a + 1) * 128], f"vt_{qc}")
                vt_sb = work.tile([128, CH], BF, name=f"vts{qc}", tag="vt",
                                  bufs=2)
                nc.scalar.copy(vt_sb[:], vt_ps[:])
                vnat = ktv.tile([128, CH], BF, name=f"v{qc}", tag="ktv")
                for s in range(nS):
                    nc.sync.dma_start_transpose(
                        vnat[:, s * 128:(s + 1) * 128],
                        vt_sb[:, s * 128:(s + 1) * 128])
                v_tiles.append(vnat)

                def rope(src_sb, dst, nm):
                    # pair-swap partitions via two strided SBUF->SBUF DMAs
                    sw = work.tile([128, CH], BF, name=f"sw_{nm}", tag="sw",
                                   bufs=4)
                    sr = src_sb[:].rearrange("(i j) t -> i j t", j=2)
                    dr = sw[:].rearrange("(i j) t -> i j t", j=2)
                    # ACT's hwdge queue: keeps rope off the SP queue, which
                    # carries the big x-prefetch transfers.
                    nc.scalar.dma_start(dr[:, 0, :], sr[:, 1, :])
                    nc.scalar.dma_start(dr[:, 1, :], sr[:, 0, :])
                    t1 = work.tile([128, CH], BF, name=f"r1_{nm}", tag="rt",
                                   bufs=4)
                    nc.vector.tensor_mul(t1[:], src_sb[:], cos_t[:])
                    t2 = work.tile([128, CH], BF, name=f"r2_{nm}", tag="rt",
                                   bufs=4)
                    nc.vector.tensor_mul(t2[:], sw[:], sin_t[:])
                    nc.vector.tensor_add(dst, t1[:], t2[:])

                qr0 = work.tile([128, CH], BF, name=f"qr0_{qc}", tag="qr",
                                bufs=4)
                qr1 = work.tile([128, CH], BF, name=f"qr1_{qc}", tag="qr",
                                bufs=4)
                ktt = ktv.tile([128, CH], BF, name=f"kt{qc}", tag="ktv")
                rope(q0_sb, qr0[:], f"q0_{qc}")
                rope(q1_sb, qr1[:], f"q1_{qc}")
                rope(k_sb, ktt[:], f"k_{qc}")
                kt_tiles.append(ktt)
                return qr0, qr1

            wo_flip = [0]

            def wo_items_for(qc, cn0, cn1, tail=False):
                t0 = qc * CH
                items = []
                for s in range(nS):
                    for n in range(nN):
                        def emit(s=s, n=n):
                            w_ps = psA.tile([128, 512], FP,
                                            name=f"w{qc}_{s}_{n}", tag="acc")
                            nc.tensor.matmul(
                                w_ps[:], cn0[:, s * 128:(s + 1) * 128],
                                wo_sb[:, n * 512:(n + 1) * 512],
                                start=True, stop=False)
                            nc.tensor.matmul(
                                w_ps[:], cn1[:, s * 128:(s + 1) * 128],
                                wo_sb[:, D + n * 512:D + (n + 1) * 512],
                                start=False, stop=True)
                            osb = work.tile([128, 512], BF,
                                            name=f"o{qc}_{s}_{n}", tag="osb",
                                            bufs=12)
                            if tail and wo_flip[0] % 2 == 1:
                                nc.scalar.copy(osb[:], w_ps[:])
                            else:
                                nc.vector.tensor_copy(osb[:], w_ps[:])
                            wo_flip[0] += 1
                            nc.sync.dma_start(
                                out[t0 + s * 128:t0 + (s + 1) * 128,
                                    n * 512:(n + 1) * 512], osb[:])
                        items.append(emit)
                return items

            pending_wo = []
            q0_next = proj_q0(0)
            qr = proj_rest(0, q0_next)
            for qc in range(nT):
                if qc + 2 < nT:
                    prefetch(qc + 2)
                qr0, qr1 = qr
                nkp = (qc + 1) * 2          # key-tile pairs per head
                nkt = nkp * 2               # 128-key tiles per head
                cns = []
                accs = {}
                ctxs = {}
                deferred = [None]           # (h, use, p)

                def emit_norm(h):
                    lp = psA.tile([128, CH], FP, name=f"lp{qc}_{h}",
                                  tag="acc")
                    acc = accs[h]
                    nc.tensor.matmul(lp[:], ones_sb[:], acc[:, 0:CH],
                                     start=True, stop=False)
                    nc.tensor.matmul(lp[:], ones_sb[:], acc[:, CH:2 * CH],
                                     start=False, stop=True)
                    ln_t = work.tile([128, CH], FP, name=f"ln{qc}_{h}",
                                     tag="r", bufs=4)
                    nc.scalar.activation(ln_t[:], lp[:],
                                         mybir.ActivationFunctionType.Ln)
                    r_bc = work.tile([128, CH], FP, name=f"r{qc}_{h}",
                                     tag="r", bufs=4)
                    nc.scalar.activation(r_bc[:], ln_t[:],
                                         mybir.ActivationFunctionType.Exp,
                                         scale=-1.0)
                    cn = work.tile([128, CH], BF, name=f"cn{qc}_{h}",
                                   tag="cn", bufs=4)
                    nc.vector.tensor_mul(cn[:], ctxs[h][:], r_bc[:])
                    cns.append(cn)

                def pair_segments(p):
                    """[(kt, q0, width, slab_off)] for this pair; diagonal
                    pairs (qc>=1) restrict to the causally-live q range."""
                    dp = p - 2 * qc
                    if qc == 0 or dp < 0:
                        return dp, [(2 * p, 0, CH, 0), (2 * p + 1, 0, CH, CH)]
                    if dp == 0:           # deltas 0, 1
                        return dp, [(2 * p, 0, CH, 0),
                                    (2 * p + 1, 128, 384, CH)]
                    return dp, [(2 * p, 256, 256, 0),     # deltas 2, 3
                                (2 * p + 1, 384, 128, 256)]

                def flush_deferred():
                    if deferred[0] is None:
                        return
                    h_, use_, segs_ = deferred[0]
                    deferred[0] = None
                    for kt, q0, w, off in segs_:
                        kc, ko = kt // nS, (kt % nS) * 128
                        nc.tensor.matmul(
                            ctxs[h_][:, q0:q0 + w],
                            v_tiles[kc][:, ko:ko + 128],
                            use_[:, off:off + w],
                            start=(kt == 0), stop=(kt == nkt - 1))

                gp = 0                      # global pair counter this chunk
                for h, qrh in enumerate((qr0, qr1)):
                    accs[h] = accp.tile([128, 2 * CH], BF,
                                        name=f"acc{qc}_{h}", tag="accp")
                    ctxs[h] = psC.tile([128, CH], FP, name=f"ctx{qc}_{h}",
                                       tag="ctx")
                    for p in range(nkp):
                        dp, segs = pair_segments(p)
                        W = segs[-1][3] + segs[-1][2]    # slab width
                        Sp = psS.tile([128, 2 * CH], FP,
                                      name=f"S{qc}_{h}_{p}", tag="s")
                        for kt, q0, w, off in segs:
                            kc, ko = kt // nS, (kt % nS) * 128
                            nc.tensor.matmul(
                                Sp[:, off:off + w],
                                kt_tiles[kc][:, ko:ko + 128],
                                qrh[:, q0:q0 + w],
                                start=True, stop=True)
                        Pp = pp.tile([128, 2 * CH], BF,
                                     name=f"P{qc}_{h}_{p}", tag="p")
                        nc.scalar.activation(
                            Pp[:, 0:W], Sp[:, 0:W],
                            mybir.ActivationFunctionType.Exp, scale=ISQ)
                        if dp >= 0:
                            if qc == 0:
                                msl = mask_sb[:, dp * 2 * CH:
                                              (dp + 1) * 2 * CH]
                            elif dp == 0:
                                msl = maskP_sb[:, 0:896]
                            else:
                                msl = maskP_sb[:, 896:1280]
                            Pm = pp.tile([128, 2 * CH], BF,
                                         name=f"Pm{qc}_{h}_{p}", tag="p")
                            nc.vector.tensor_mul(Pm[:, 0:W], Pp[:, 0:W], msl)
                            use = Pm
                        else:
                            use = Pp
                        if p == 0:
                            nc.vector.tensor_copy(accs[h][:], use[:])
                        elif qc == 0 or dp < 0:
                            nc.vector.tensor_add(accs[h][:], accs[h][:],
                                                 use[:])
                        else:
                            for kt, q0, w, off in segs:
                                ah = 0 if kt % 2 == 0 else CH
                                nc.vector.tensor_add(
                                    accs[h][:, ah + q0:ah + q0 + w],
                                    accs[h][:, ah + q0:ah + q0 + w],
                                    use[:, off:off + w])
                        flush_deferred()
                        deferred[0] = (h, use, segs)
                        if pending_wo and (gp % 2 == 1):
                            pending_wo.pop(0)()
                        gp += 1

                # chunk epilogue: start next chunk's first projection chain
                # before the final deferred ctx, then both heads'
                # normalizations (their ACT/DVE work lands in the ACT-idle
                # projection window instead of the exp-bound pair stream).
                if qc + 1 < nT:
                    q0_next = proj_q0(qc + 1)
                flush_deferred()
                emit_norm(0)
                emit_norm(1)
                while pending_wo:
                    pending_wo.pop(0)()
                pending_wo = wo_items_for(qc, cns[0], cns[1],
                                          tail=(qc == nT - 1))
                if qc + 1 < nT:
                    qr = proj_rest(qc + 1, q0_next)
            while pending_wo:
                pending_wo.pop(0)()

    _split_multi_waits(nc, 1)
    return nc


# --------------------------------------------------------------------------
def host_prep(x, WQ, WK, WV, WO):
    ROPE_BASE = 10000.0

    xTc = np.ascontiguousarray(
        np.asarray(x, dtype=np.float32).reshape(T, D).T).astype(BFNP)

    omega = 1.0 / (ROPE_BASE ** (np.arange(0, DH, 2, dtype=np.float64) / DH))
    ang = np.outer(omega, np.arange(T, dtype=np.float64))
    cosT = np.repeat(np.cos(ang), 2, axis=0).astype(BFNP)
    sgn = np.tile(np.array([-1.0, 1.0]), DH // 2)[:, None]
    sinT = (np.repeat(np.sin(ang), 2, axis=0) * sgn).astype(BFNP)

    p_i = np.arange(128)[:, None]
    f_i = np.arange(CH)[None, :]
    mlist = [(128 * dl + p_i <= f_i).astype(np.float32) for dl in range(nS)]
    masks = np.concatenate(mlist, axis=1).astype(BFNP)
    # packed diagonal masks for the causally-live q ranges:
    # [d0 full 512 | d1 cols 128: | d2 cols 256: | d3 cols 384:] = 1280 cols
    masksP = np.concatenate(
        [mlist[0], mlist[1][:, 128:], mlist[2][:, 256:], mlist[3][:, 384:]],
        axis=1).astype(BFNP)

    def tile_pmaj(w, ncols):
        return np.ascontiguousarray(
            np.asarray(w, dtype=np.float32).reshape(nA, 128, ncols)
            .transpose(1, 0, 2).reshape(128, nA * ncols)).astype(BFNP)

    in_maps = []
    for c in range(N_CORES):
        kv = c // 2
        wo_c = np.asarray(WO, dtype=np.float32)[256 * c:256 * (c + 1), :]
        in_maps.append({
            "xT": xTc,
            "wq2": tile_pmaj(np.asarray(WQ)[:, 256 * c:256 * (c + 1)], 256),
            "wk2": tile_pmaj(np.asarray(WK)[:, 128 * kv:128 * (kv + 1)], 128),
            "wv2": tile_pmaj(np.asarray(WV)[:, 128 * kv:128 * (kv + 1)], 128),
            "wo2": np.ascontiguousarray(
                wo_c.reshape(2, 128, D).transpose(1, 0, 2)
                .reshape(128, 2 * D)).astype(BFNP),
            "cosT": cosT, "sinT": sinT, "masks": masks,
            "masksP": masksP,
            "onesM": np.ones((128, 128), dtype=BFNP),
        })
    return in_maps


_NC_CACHE = {}


def _get_nc():
    if "nc" not in _NC_CACHE:
        _NC_CACHE["nc"] = build_nc()
    return _NC_CACHE["nc"]


def run_on_hw(inputs, trace=False):
    """Returns (out [1,T,D] fp32, BassKernelResults)."""
    nc = _get_nc()
    in_maps = host_prep(inputs["x"], inputs["WQ"], inputs["WK"],
                        inputs["WV"], inputs["WO"])
    res = run_bass_kernel_spmd(nc, in_maps, list(range(N_CORES)),
                               trace=trace)
    acc = np.zeros((T, D), dtype=np.float32)
    for c in range(N_CORES):
        acc += res.results[c]["out"].astype(np.float32)
    return acc[None], res


def kernel(x, WQ, WK, WV, WO):
    out, _ = run_on_hw({"x": x, "WQ": WQ, "WK": WK, "WV": WV, "WO": WO})
    return out


# revision 19
# speedup vs baseline: 1.0536x; 1.0144x over previous
"""GroupedQueryAttention (B=1, T=4096, D=2048, 16 heads / 4 kv heads, RoPE,
causal) on 8 Trainium2 NeuronCores.

Sharding: head tensor-parallel. Core c owns q-heads {2c, 2c+1} and kv head
c//2 (WQ/WO split along head dim, WK/WV along kv-head dim). Each core
computes its partial out = ctx_heads @ WO_slice over the full sequence;
partials are summed across cores on the host gather side.

v3 pipeline:
 - softmax denominator: per-pair DVE accumulation of P into a bf16 running
   sum + one ones-matrix matmul per (head, chunk) that also broadcasts the
   sums across partitions; reciprocal via ACT exp(-Ln(l)) (same table set
   as the softmax Exp, so no ACT table switches).
 - exp batched over key-tile PAIRS: one ACT instruction per [128,1024]
   PSUM S-pair (amortizes the 352-cycle ACT fixed overhead).
 - V produced transposed (weight-stationary chains, N=512 streams) then
   de-transposed with dma_start_transpose (XBAR).
 - flat cross-head pair stream with one-pair-deferred ctx emission and
   deferred per-head normalization, so the PE never drains at head or
   chunk boundaries (keeps HAM at full clock).
 - WO matmuls of chunk qc interleaved into chunk qc+1's attention stream;
   PSUM->SBUF output copies alternate between ACT and DVE.
 - split weight/x DMAs + 2-chunk-ahead prefetch to cut startup and SP
   queue head-of-line blocking; bf16 rope; bf16 partial-output DMA.
"""

import math

import numpy as np
import ml_dtypes

import concourse.bass as bass
import concourse.mybir as mybir
import concourse.tile as tile
from concourse.bass_utils import run_bass_kernel_spmd

FP = mybir.dt.float32
BF = mybir.dt.bfloat16
BFNP = ml_dtypes.bfloat16

T, D, DH = 4096, 2048, 128      # seq len, model dim, head dim
CH = 512                        # query-chunk (free dim of attention matmuls)
N_CORES = 8

nT = T // CH          # 8 q-chunks
nA = D // 128         # 16 contraction tiles
nS = CH // 128        # 4 128-subtiles per chunk
nN = D // 512         # 4 output column tiles
ISQ = 1.0 / math.sqrt(float(DH))


# --------------------------------------------------------------------------
# workaround: this walrus build rejects instructions carrying >1 sem-waits
# (setupSyncWait "Too many sync wait commands"); split extras into NoOps.
_WS_CTR = [0]


def _split_multi_waits(nc, limit=1):
    for f in nc.m.functions:
        for bb in f.blocks:
            il = bb.instructions
            i = 0
            while i < len(il):
                inst = il[i]
                si = getattr(inst, "sync_info", None)
                if si is not None and len(si.on_wait) > limit:
                    waits = list(si.on_wait)
                    keep, rest = waits[:limit], waits[limit:]
                    nops = []
                    for j in range(0, len(rest), limit):
                        _WS_CTR[0] += 1
                        n = mybir.InstNoOp(name=f"waitsplit-{_WS_CTR[0]}")
                        n.engine = inst.engine
                        n.sync_info = mybir.SyncInfo(
                            on_wait=rest[j:j + limit], on_update=[])
                        nops.append(n)
                    inst.sync_info = mybir.SyncInfo(
                        on_wait=keep, on_update=list(si.on_update))
                    for k, n in enumerate(nops):
                        il.insert(i + k, n)
                    i += len(nops)
                i += 1


# --------------------------------------------------------------------------
def build_nc():
    nc = bass.Bass()

    xT = nc.dram_tensor("xT", [D, T], BF, kind="ExternalInput")
    wq2 = nc.dram_tensor("wq2", [128, nA * 256], BF, kind="ExternalInput")
    wk2 = nc.dram_tensor("wk2", [128, nA * 128], BF, kind="ExternalInput")
    wv2 = nc.dram_tensor("wv2", [128, nA * 128], BF, kind="ExternalInput")
    wo2 = nc.dram_tensor("wo2", [128, 2 * D], BF, kind="ExternalInput")
    cosT = nc.dram_tensor("cosT", [128, T], BF, kind="ExternalInput")
    sinT = nc.dram_tensor("sinT", [128, T], BF, kind="ExternalInput")
    masks = nc.dram_tensor("masks", [128, nS * CH], BF, kind="ExternalInput")
    masksP = nc.dram_tensor("masksP", [128, 1280], BF, kind="ExternalInput")
    onesM = nc.dram_tensor("onesM", [128, 128], BF, kind="ExternalInput")
    out = nc.dram_tensor("out", [T, D], BF, kind="ExternalOutput")

    xTr = xT.rearrange("(a p) t -> p a t", p=128)

    with tile.TileContext(nc) as tc:
        with (
            tc.tile_pool(name="res", bufs=1) as res,
            tc.tile_pool(name="ktv", bufs=2 * nT) as ktv,
            tc.tile_pool(name="xt", bufs=3) as xtp,
            tc.tile_pool(name="tab", bufs=6) as tab,
            tc.tile_pool(name="work", bufs=2) as work,
            tc.tile_pool(name="pp", bufs=6) as pp,
            tc.tile_pool(name="accp", bufs=2) as accp,
            tc.tile_pool(name="psS", bufs=2, space="PSUM") as psS,
            tc.tile_pool(name="psC", bufs=2, space="PSUM") as psC,
            tc.tile_pool(name="psA", bufs=2, space="PSUM") as psA,
        ):
            wq_sb = res.tile([128, nA * 256], BF, name="wq_sb")
            wk_sb = res.tile([128, nA * 128], BF, name="wk_sb")
            wv_sb = res.tile([128, nA * 128], BF, name="wv_sb")
            wo_sb = res.tile([128, 2 * D], BF, name="wo_sb")
            mask_sb = res.tile([128, nS * CH], BF, name="mask_sb")
            maskP_sb = res.tile([128, 1280], BF, name="maskP_sb")
            ones_sb = res.tile([128, 128], BF, name="ones_sb")

            def wq_part(part):
                c0, c1 = part * nA * 64, (part + 1) * nA * 64
                nc.sync.dma_start(wq_sb[:, c0:c1], wq2[:, c0:c1])

            def wkv_part(w_sb, w_dram, part):
                c0, c1 = part * nA * 64, (part + 1) * nA * 64
                nc.sync.dma_start(w_sb[:, c0:c1], w_dram[:, c0:c1])

            kt_tiles = []
            v_tiles = []
            x_tiles = {}
            cs_tiles = {}

            def prefetch(qc):
                t0 = qc * CH
                xa = xtp.tile([128, nA * CH], BF, name=f"x{qc}", tag="x")
                for part in range(2):
                    a0, a1 = part * (nA // 2), (part + 1) * (nA // 2)
                    nc.sync.dma_start(
                        xa[:, a0 * CH:a1 * CH].rearrange(
                            "p (a t) -> p a t", a=nA // 2),
                        xTr[:, a0:a1, t0:t0 + CH])
                x_tiles[qc] = xa
                cos_t = tab.tile([128, CH], BF, name=f"cos{qc}", tag="tab")
                sin_t = tab.tile([128, CH], BF, name=f"sin{qc}", tag="tab")
                nc.sync.dma_start(cos_t[:], cosT[:, t0:t0 + CH])
                nc.sync.dma_start(sin_t[:], sinT[:, t0:t0 + CH])
                cs_tiles[qc] = (cos_t, sin_t)

            # chunk-0 x tiles at per-a granularity, interleaved just-in-time
            # with the weight pieces the q0 chain consumes alongside them.
            xa0 = xtp.tile([128, nA * CH], BF, name="x0", tag="x")

            def xa0_part(a):
                nc.sync.dma_start(xa0[:, a * CH:(a + 1) * CH],
                                  xTr[:, a, 0:CH])

            x_tiles[0] = xa0
            wq_part(0)
            for a in (0, 1, 2, 3):
                xa0_part(a)
            wq_part(1)
            for a in (4, 5, 6, 7):
                xa0_part(a)
            wq_part(2)
            for a in (8, 9, 10, 11):
                xa0_part(a)
            wq_part(3)
            for a in (12, 13, 14, 15):
                xa0_part(a)
            for part in range(2):
                wkv_part(wk_sb, wk2, part)
                wkv_part(wv_sb, wv2, part)
            cos0 = tab.tile([128, CH], BF, name="cos0", tag="tab")
            sin0 = tab.tile([128, CH], BF, name="sin0", tag="tab")
            nc.sync.dma_start(cos0[:], cosT[:, 0:CH])
            nc.sync.dma_start(sin0[:], sinT[:, 0:CH])
            cs_tiles[0] = (cos0, sin0)
            nc.sync.dma_start(mask_sb[:], masks[:])
            nc.sync.dma_start(maskP_sb[:], masksP[:])
            nc.sync.dma_start(ones_sb[:], onesM[:])
            for part in range(2):
                c0, c1 = part * D, (part + 1) * D
                nc.sync.dma_start(wo_sb[:, c0:c1], wo2[:, c0:c1])
            prefetch(1)

            def proj_chain(xa, lhs_fn, nm):
                acc = psA.tile([128, CH], FP, name=f"ps_{nm}", tag="acc")
                for a in range(nA):
                    nc.tensor.matmul(
                        acc[:], lhs_fn(a), xa[:, a * CH:(a + 1) * CH],
                        start=(a == 0), stop=(a == nA - 1))
                return acc

            def proj_q0(qc):
                xa = x_tiles[qc]
                q0_ps = proj_chain(
                    xa, lambda a: wq_sb[:, a * 256:a * 256 + 128], f"q0_{qc}")
                q0_sb = work.tile([128, CH], BF, name=f"q0s{qc}", tag="qk",
                                  bufs=4)
                nc.scalar.copy(q0_sb[:], q0_ps[:])
                return q0_sb

            def proj_rest(qc, q0_sb):
                xa = x_tiles.pop(qc)
                cos_t, sin_t = cs_tiles.pop(qc)

                q1_ps = proj_chain(
                    xa, lambda a: wq_sb[:, a * 256 + 128:a * 256 + 256],
                    f"q1_{qc}")
                q1_sb = work.tile([128, CH], BF, name=f"q1s{qc}", tag="qk",
                                  bufs=4)
                nc.scalar.copy(q1_sb[:], q1_ps[:])

                k_ps = proj_chain(
                    xa, lambda a: wk_sb[:, a * 128:(a + 1) * 128], f"k_{qc}")
                k_sb = work.tile([128, CH], BF, name=f"ks{qc}", tag="qk",
                                 bufs=4)
                nc.scalar.copy(k_sb[:], k_ps[:])

                # V transposed (weight-stationary, full-width streams), then
                # de-transpose via the DMA XBAR into natural [t, dh] layout.
                vt_ps = proj_chain(
                    xa, lambda a: wv_sb[:, a * 128:(a + 1) * 128], f"vt_{qc}")
                vt_sb = work.tile([128, CH], BF, name=f"vts{qc}", tag="vt",
                                  bufs=2)
                nc.scalar.copy(vt_sb[:], vt_ps[:])
                vnat = ktv.tile([128, CH], BF, name=f"v{qc}", tag="ktv")
                for s in range(nS):
                    nc.sync.dma_start_transpose(
                        vnat[:, s * 128:(s + 1) * 128],
                        vt_sb[:, s * 128:(s + 1) * 128])
                v_tiles.append(vnat)

                def rope(src_sb, dst, nm):
                    # pair-swap partitions via two strided SBUF->SBUF DMAs
                    sw = work.tile([128, CH], BF, name=f"sw_{nm}", tag="sw",
                                   bufs=4)
                    sr = src_sb[:].rearrange("(i j) t -> i j t", j=2)
                    dr = sw[:].rearrange("(i j) t -> i j t", j=2)
                    # ACT's hwdge queue: keeps rope off the SP queue, which
                    # carries the big x-prefetch transfers.
                    nc.scalar.dma_start(dr[:, 0, :], sr[:, 1, :])
                    nc.scalar.dma_start(dr[:, 1, :], sr[:, 0, :])
                    t1 = work.tile([128, CH], BF, name=f"r1_{nm}", tag="rt",
                                   bufs=4)
                    nc.vector.tensor_mul(t1[:], src_sb[:], cos_t[:])
                    t2 = work.tile([128, CH], BF, name=f"r2_{nm}", tag="rt",
                                   bufs=4)
                    nc.vector.tensor_mul(t2[:], sw[:], sin_t[:])
                    nc.vector.tensor_add(dst, t1[:], t2[:])

                qr0 = work.tile([128, CH], BF, name=f"qr0_{qc}", tag="qr",
                                bufs=4)
                qr1 = work.tile([128, CH], BF, name=f"qr1_{qc}", tag="qr",
                                bufs=4)
                ktt = ktv.tile([128, CH], BF, name=f"kt{qc}", tag="ktv")
                rope(q0_sb, qr0[:], f"q0_{qc}")
                rope(q1_sb, qr1[:], f"q1_{qc}")
                rope(k_sb, ktt[:], f"k_{qc}")
                kt_tiles.append(ktt)
                return qr0, qr1

            wo_flip = [0]

            def wo_items_for(qc, cn0, cn1, tail=False):
                t0 = qc * CH
                items = []
                for s in range(nS):
                    for n in range(nN):
                        def emit(s=s, n=n):
                            w_ps = psA.tile([128, 512], FP,
                                            name=f"w{qc}_{s}_{n}", tag="acc")
                            nc.tensor.matmul(
                                w_ps[:], cn0[:, s * 128:(s + 1) * 128],
                                wo_sb[:, n * 512:(n + 1) * 512],
                                start=True, stop=False)
                            nc.tensor.matmul(
                                w_ps[:], cn1[:, s * 128:(s + 1) * 128],
                                wo_sb[:, D + n * 512:D + (n + 1) * 512],
                                start=False, stop=True)
                            osb = work.tile([128, 512], BF,
                                            name=f"o{qc}_{s}_{n}", tag="osb",
                                            bufs=12)
                            if tail and wo_flip[0] % 2 == 1:
                                nc.scalar.copy(osb[:], w_ps[:])
                            else:
                                nc.vector.tensor_copy(osb[:], w_ps[:])
                            wo_flip[0] += 1
                            nc.sync.dma_start(
                                out[t0 + s * 128:t0 + (s + 1) * 128,
                                    n * 512:(n + 1) * 512], osb[:])
                        items.append(emit)
                return items

            pending_wo = []
            q0_next = proj_q0(0)
            qr = proj_rest(0, q0_next)
            for qc in range(nT):
                if qc + 2 < nT:
                    prefetch(qc + 2)
                # early chunks: attention windows are too short to hide the
                # next chunk's projection+rope dependency chain - emit the
                # whole projection up front instead of sandwiching it.
                early = qc < 2 and qc + 1 < nT
                if early:
                    qr_next = proj_rest(qc + 1, proj_q0(qc + 1))
                qr0, qr1 = qr
                nkp = (qc + 1) * 2          # key-tile pairs per head
                nkt = nkp * 2               # 128-key tiles per head
                cns = []
                accs = {}
                ctxs = {}
                deferred = [None]           # (h, use, p)

                def emit_norm(h):
                    lp = psA.tile([128, CH], FP, name=f"lp{qc}_{h}",
                                  tag="acc")
                    acc = accs[h]
                    nc.tensor.matmul(lp[:], ones_sb[:], acc[:, 0:CH],
                                     start=True, stop=False)
                    nc.tensor.matmul(lp[:], ones_sb[:], acc[:, CH:2 * CH],
                                     start=False, stop=True)
                    ln_t = work.tile([128, CH], FP, name=f"ln{qc}_{h}",
                                     tag="r", bufs=4)
                    nc.scalar.activation(ln_t[:], lp[:],
                                         mybir.ActivationFunctionType.Ln)
                    r_bc = work.tile([128, CH], FP, name=f"r{qc}_{h}",
                                     tag="r", bufs=4)
                    nc.scalar.activation(r_bc[:], ln_t[:],
                                         mybir.ActivationFunctionType.Exp,
                                         scale=-1.0)
                    cn = work.tile([128, CH], BF, name=f"cn{qc}_{h}",
                                   tag="cn", bufs=4)
                    nc.vector.tensor_mul(cn[:], ctxs[h][:], r_bc[:])
                    cns.append(cn)

                def pair_segments(p):
                    """[(kt, q0, width, slab_off)] for this pair; diagonal
                    pairs (qc>=1) restrict to the causally-live q range."""
                    dp = p - 2 * qc
                    if qc == 0 or dp < 0:
                        return dp, [(2 * p, 0, CH, 0), (2 * p + 1, 0, CH, CH)]
                    if dp == 0:           # deltas 0, 1
                        return dp, [(2 * p, 0, CH, 0),
                                    (2 * p + 1, 128, 384, CH)]
                    return dp, [(2 * p, 256, 256, 0),     # deltas 2, 3
                                (2 * p + 1, 384, 128, 256)]

                def flush_deferred():
                    if deferred[0] is None:
                        return
                    h_, use_, segs_ = deferred[0]
                    deferred[0] = None
                    for kt, q0, w, off in segs_:
                        kc, ko = kt // nS, (kt % nS) * 128
                        nc.tensor.matmul(
                            ctxs[h_][:, q0:q0 + w],
                            v_tiles[kc][:, ko:ko + 128],
                            use_[:, off:off + w],
                            start=(kt == 0), stop=(kt == nkt - 1))

                gp = 0                      # global pair counter this chunk
                for h, qrh in enumerate((qr0, qr1)):
                    accs[h] = accp.tile([128, 2 * CH], BF,
                                        name=f"acc{qc}_{h}", tag="accp")
                    ctxs[h] = psC.tile([128, CH], FP, name=f"ctx{qc}_{h}",
                                       tag="ctx")
                    for p in range(nkp):
                        dp, segs = pair_segments(p)
                        W = segs[-1][3] + segs[-1][2]    # slab width
                        Sp = psS.tile([128, 2 * CH], FP,
                                      name=f"S{qc}_{h}_{p}", tag="s")
                        for kt, q0, w, off in segs:
                            kc, ko = kt // nS, (kt % nS) * 128
                            nc.tensor.matmul(
                                Sp[:, off:off + w],
                                kt_tiles[kc][:, ko:ko + 128],
                                qrh[:, q0:q0 + w],
                                start=True, stop=True)
                        Pp = pp.tile([128, 2 * CH], BF,
                                     name=f"P{qc}_{h}_{p}", tag="p")
                        nc.scalar.activation(
                            Pp[:, 0:W], Sp[:, 0:W],
                            mybir.ActivationFunctionType.Exp, scale=ISQ)
                        if dp >= 0:
                            if qc == 0:
                                msl = mask_sb[:, dp * 2 * CH:
                                              (dp + 1) * 2 * CH]
                            elif dp == 0:
                                msl = maskP_sb[:, 0:896]
                            else:
                                msl = maskP_sb[:, 896:1280]
                            Pm = pp.tile([128, 2 * CH], BF,
                                         name=f"Pm{qc}_{h}_{p}", tag="p")
                            nc.vector.tensor_mul(Pm[:, 0:W], Pp[:, 0:W], msl)
                            use = Pm
                        else:
                            use = Pp
                        if p == 0:
                            nc.vector.tensor_copy(accs[h][:], use[:])
                        elif qc == 0 or dp < 0:
                            nc.vector.tensor_add(accs[h][:], accs[h][:],
                                                 use[:])
                        else:
                            for kt, q0, w, off in segs:
                                ah = 0 if kt % 2 == 0 else CH
                                nc.vector.tensor_add(
                                    accs[h][:, ah + q0:ah + q0 + w],
                                    accs[h][:, ah + q0:ah + q0 + w],
                                    use[:, off:off + w])
                        flush_deferred()
                        deferred[0] = (h, use, segs)
                        if pending_wo and (gp % 2 == 1):
                            pending_wo.pop(0)()
                        gp += 1

                # chunk epilogue: start next chunk's first projection chain
                # before the final deferred ctx, then both heads'
                # normalizations (their ACT/DVE work lands in the ACT-idle
                # projection window instead of the exp-bound pair stream).
                if not early and qc + 1 < nT:
                    q0_next = proj_q0(qc + 1)
                flush_deferred()
                emit_norm(0)
                emit_norm(1)
                while pending_wo:
                    pending_wo.pop(0)()
                pending_wo = wo_items_for(qc, cns[0], cns[1],
                                          tail=(qc == nT - 1))
                if early:
                    qr = qr_next
                elif qc + 1 < nT:
                    qr = proj_rest(qc + 1, q0_next)
            while pending_wo:
                pending_wo.pop(0)()

    _split_multi_waits(nc, 1)
    return nc


# --------------------------------------------------------------------------
def host_prep(x, WQ, WK, WV, WO):
    ROPE_BASE = 10000.0

    xTc = np.ascontiguousarray(
        np.asarray(x, dtype=np.float32).reshape(T, D).T).astype(BFNP)

    omega = 1.0 / (ROPE_BASE ** (np.arange(0, DH, 2, dtype=np.float64) / DH))
    ang = np.outer(omega, np.arange(T, dtype=np.float64))
    cosT = np.repeat(np.cos(ang), 2, axis=0).astype(BFNP)
    sgn = np.tile(np.array([-1.0, 1.0]), DH // 2)[:, None]
    sinT = (np.repeat(np.sin(ang), 2, axis=0) * sgn).astype(BFNP)

    p_i = np.arange(128)[:, None]
    f_i = np.arange(CH)[None, :]
    mlist = [(128 * dl + p_i <= f_i).astype(np.float32) for dl in range(nS)]
    masks = np.concatenate(mlist, axis=1).astype(BFNP)
    # packed diagonal masks for the causally-live q ranges:
    # [d0 full 512 | d1 cols 128: | d2 cols 256: | d3 cols 384:] = 1280 cols
    masksP = np.concatenate(
        [mlist[0], mlist[1][:, 128:], mlist[2][:, 256:], mlist[3][:, 384:]],
        axis=1).astype(BFNP)

    def tile_pmaj(w, ncols):
        return np.ascontiguousarray(
            np.asarray(w, dtype=np.float32).reshape(nA, 128, ncols)
            .transpose(1, 0, 2).reshape(128, nA * ncols)).astype(BFNP)

    in_maps = []
    for c in range(N_CORES):
        kv = c // 2
        wo_c = np.asarray(WO, dtype=np.float32)[256 * c:256 * (c + 1), :]
        in_maps.append({
            "xT": xTc,
            "wq2": tile_pmaj(np.asarray(WQ)[:, 256 * c:256 * (c + 1)], 256),
            "wk2": tile_pmaj(np.asarray(WK)[:, 128 * kv:128 * (kv + 1)], 128),
            "wv2": tile_pmaj(np.asarray(WV)[:, 128 * kv:128 * (kv + 1)], 128),
            "wo2": np.ascontiguousarray(
                wo_c.reshape(2, 128, D).transpose(1, 0, 2)
                .reshape(128, 2 * D)).astype(BFNP),
            "cosT": cosT, "sinT": sinT, "masks": masks,
            "masksP": masksP,
            "onesM": np.ones((128, 128), dtype=BFNP),
        })
    return in_maps


_NC_CACHE = {}


def _get_nc():
    if "nc" not in _NC_CACHE:
        _NC_CACHE["nc"] = build_nc()
    return _NC_CACHE["nc"]


def run_on_hw(inputs, trace=False):
    """Returns (out [1,T,D] fp32, BassKernelResults)."""
    nc = _get_nc()
    in_maps = host_prep(inputs["x"], inputs["WQ"], inputs["WK"],
                        inputs["WV"], inputs["WO"])
    res = run_bass_kernel_spmd(nc, in_maps, list(range(N_CORES)),
                               trace=trace)
    acc = np.zeros((T, D), dtype=np.float32)
    for c in range(N_CORES):
        acc += res.results[c]["out"].astype(np.float32)
    return acc[None], res


def kernel(x, WQ, WK, WV, WO):
    out, _ = run_on_hw({"x": x, "WQ": WQ, "WK": WK, "WV": WV, "WO": WO})
    return out
